# revision 2
# baseline (speedup 1.0000x reference)
"""DiceBCE + OHNM loss for Trainium2 (8 NeuronCores, SPMD data-parallel over batch).

Device side (Bass/Tile, one launch, core b handles batch element b):
  reads preds[b] (f32) + targs[b] (i32), computes the elementwise BCE map
      p    = sigmoid(x)
      loss = max(p,0) - p*t + log1p(exp(-|p|))      (p in (0,1))
           = softplus(p * (1 - 2t))                  (exact restructure, t in {0,1})
  and writes the full loss map back to HBM.

Host side (data-dependent glue, mirrors the reference's host-side numpy):
  top-k hard-negative selection over the device loss map, positive gather,
  seeded-RNG padding, then the tiny dice + mean reductions over the ~336k
  selected elements.
"""

import numpy as np

B, C, D, H, W = 8, 1, 128, 128, 128
P = 128
FREE = (C * D * H * W) // P  # 16384 elements per partition per core
TILE_W = 2048
EPS = 1e-10
OHNM_RATIO = 3
DEFAULT_NEG_PERC = 0.1

_CACHE = {}


def _build_nc():
    import concourse.bass as bass
    import concourse.tile as tile
    from concourse import bacc, mybir

    nc = bacc.Bacc("TRN2", target_bir_lowering=False, debug=False, num_devices=B)
    x = nc.dram_tensor("preds", [P, FREE], mybir.dt.float32, kind="ExternalInput").ap()
    t = nc.dram_tensor("targs", [P, FREE], mybir.dt.int32, kind="ExternalInput").ap()
    l = nc.dram_tensor("loss", [P, FREE], mybir.dt.float32, kind="ExternalOutput").ap()

    n_tiles = FREE // TILE_W
    # softplus(z) = -ln(sigmoid(-z)); with m = (t - 0.5)*p and z = p(1-2t) = -2m:
    #   loss = softplus(p(1-2t)) = -ln(sigmoid(2m))
    # Phase A uses the sigmoid ACT table, phase B the natural-log table; keeping
    # them in separate phases means a single table switch on the ACT engine.
    with tile.TileContext(nc) as tc:
        with (
            tc.tile_pool(name="io", bufs=3) as io,
            tc.tile_pool(name="tmp", bufs=2) as tmp,
            tc.tile_pool(name="qbuf", bufs=1) as qbuf,
        ):
            q = qbuf.tile([P, FREE], mybir.dt.float32, tag="q")
            for i in range(n_tiles):
                sl = bass.ts(i, TILE_W)
                xt = io.tile([P, TILE_W], mybir.dt.float32, tag="x")
                nc.sync.dma_start(xt[:], x[:, sl])
                tt = io.tile([P, TILE_W], mybir.dt.int32, tag="t")
                nc.sync.dma_start(tt[:], t[:, sl])

                pt = tmp.tile([P, TILE_W], mybir.dt.float32, tag="p")
                nc.scalar.activation(pt[:], xt[:], mybir.ActivationFunctionType.Sigmoid)
                # m = (t - 0.5) * p
                mt = tmp.tile([P, TILE_W], mybir.dt.float32, tag="m")
                nc.vector.scalar_tensor_tensor(
                    mt[:], tt[:], 0.5, pt[:],
                    mybir.AluOpType.subtract, mybir.AluOpType.mult,
                )
                # q = sigmoid(2m)
                nc.scalar.activation(
                    q[:, sl], mt[:], mybir.ActivationFunctionType.Sigmoid, scale=2.0
                )
            for i in range(n_tiles):
                sl = bass.ts(i, TILE_W)
                nlt = tmp.tile([P, TILE_W], mybir.dt.float32, tag="nl")
                nc.scalar.activation(nlt[:], q[:, sl], mybir.ActivationFunctionType.Ln)
                lt = io.tile([P, TILE_W], mybir.dt.float32, tag="l")
                nc.vector.tensor_scalar_mul(lt[:], nlt[:], -1.0)
                nc.sync.dma_start(l[:, sl], lt[:])
    nc.compile()
    return nc


def _get_nc():
    if "nc" not in _CACHE:
        _CACHE["nc"] = _build_nc()
    return _CACHE["nc"]


def run_device(preds, targs, trace=False):
    """Run the SPMD bass kernel on cores 0..7; returns (losses_full, BassKernelResults)."""
    from concourse.bass_utils import run_bass_kernel_spmd

    nc = _get_nc()
    in_maps = []
    for b in range(B):
        in_maps.append({
            "preds": np.ascontiguousarray(preds[b].reshape(P, FREE), dtype=np.float32),
            "targs": np.ascontiguousarray(targs[b].reshape(P, FREE), dtype=np.int32),
        })
    res = run_bass_kernel_spmd(nc, in_maps, core_ids=list(range(B)), trace=trace)
    losses = np.stack([res.results[b]["loss"] for b in range(B)])
    return losses.reshape(B, C, D, H, W), res


def _host_finish(preds, targs, losses):
    """Mirror of the reference's host-side get_idxs/pad + dice/mean reductions."""
    x = np.asarray(preds).reshape(-1)
    t = np.asarray(targs).reshape(-1)
    l = np.asarray(losses).reshape(-1)
    numel = t.size
    n_pos = int(t.sum())
    n_neg = numel - n_pos
    if n_pos == 0:
        n_hns = int(DEFAULT_NEG_PERC * n_neg)
    else:
        n_hns = min(n_pos * OHNM_RATIO, n_neg)

    neg_losses = l[t == 0]
    if n_hns > 0:
        if n_hns < neg_losses.size:
            part = np.argpartition(-neg_losses, n_hns - 1)[:n_hns]
        else:
            part = np.arange(neg_losses.size)
        hns_idxs = part[np.argsort(-neg_losses[part])]
    else:
        hns_idxs = np.empty(0, dtype=np.int64)
    pos_idxs = np.nonzero(t == 1)[0]
    idxs = np.concatenate([hns_idxs, pos_idxs]).astype(np.int64)
    n_needed = len(idxs) % (B * C)
    if n_needed != 0:
        mask = np.ones(numel, dtype=bool)
        mask[idxs] = False
        remaining = np.nonzero(mask)[0]
        w = remaining.astype(np.float64)
        rng = np.random.default_rng(0)
        extra = rng.choice(remaining, size=n_needed, replace=False, p=w / w.sum())
        idxs = np.concatenate([idxs, extra.astype(np.int64)])

    x_sel = x[idxs].astype(np.float64)
    p_sel = 1.0 / (1.0 + np.exp(-x_sel))          # sigmoid(preds) at selected
    t_sel = t[idxs].astype(np.float64)
    loss_sel = l[idxs].astype(np.float64)

    p2 = (1.0 / (1.0 + np.exp(-p_sel))).reshape(B * C, -1)   # dice re-sigmoids
    ts = t_sel.reshape(B * C, -1)
    inter = (p2 * ts).sum(axis=1)
    denom = p2.sum(axis=1) + ts.sum(axis=1)
    dice = np.mean(1.0 - (2.0 * inter + EPS) / (denom + EPS))
    return np.float32(dice + loss_sel.mean())


def kernel(preds, targs):
    preds = np.asarray(preds, dtype=np.float32)
    targs = np.asarray(targs, dtype=np.int32)
    assert preds.shape == (B, C, D, H, W) and targs.shape == (B, C, D, H, W)
    losses, _ = run_device(preds, targs, trace=False)
    return _host_finish(preds, targs, losses)


# revision 3
# speedup vs baseline: 1.1416x; 1.1416x over previous
"""DiceBCE + OHNM loss for Trainium2 (8 NeuronCores, SPMD data-parallel over batch).

Device side (Bass/Tile, one launch, core b handles batch element b):
  reads preds[b] (f32) and computes the negative-class BCE map
      p  = sigmoid(x)
      nl = softplus(p) = -ln(sigmoid(-p))         (the BCE loss where t == 0)
  writing nl back to HBM. targs never touches the device: the loss at the
  sparse positive sites (~0.5%) is patched on host via the exact identity
      loss|t=1 = softplus(p) - p = nl - p.

Host side (data-dependent glue, mirrors the reference's host-side numpy):
  top-k hard-negative selection over the device loss map, positive patch +
  gather, seeded-RNG padding, then the tiny dice + mean reductions over the
  ~336k selected elements.
"""

import numpy as np

B, C, D, H, W = 8, 1, 128, 128, 128
P = 128
FREE = (C * D * H * W) // P  # 16384 elements per partition per core
TILE_W = 2048
EPS = 1e-10
OHNM_RATIO = 3
DEFAULT_NEG_PERC = 0.1

_CACHE = {}


def _build_nc():
    import concourse.bass as bass
    import concourse.tile as tile
    from concourse import bacc, mybir

    nc = bacc.Bacc("TRN2", target_bir_lowering=False, debug=False, num_devices=B)
    x = nc.dram_tensor("preds", [P, FREE], mybir.dt.float32, kind="ExternalInput").ap()
    l = nc.dram_tensor("loss", [P, FREE], mybir.dt.float32, kind="ExternalOutput").ap()

    n_tiles = FREE // TILE_W
    # softplus(p) = -ln(sigmoid(-p)).  Phase A uses the sigmoid ACT table,
    # phase B the natural-log table; separate phases -> one table switch.
    with tile.TileContext(nc) as tc:
        with (
            tc.tile_pool(name="io", bufs=3) as io,
            tc.tile_pool(name="tmp", bufs=2) as tmp,
            tc.tile_pool(name="qbuf", bufs=1) as qbuf,
        ):
            q = qbuf.tile([P, FREE], mybir.dt.float32, tag="q")
            for i in range(n_tiles):
                sl = bass.ts(i, TILE_W)
                xt = io.tile([P, TILE_W], mybir.dt.float32, tag="x")
                nc.sync.dma_start(xt[:], x[:, sl])
                pt = tmp.tile([P, TILE_W], mybir.dt.float32, tag="p")
                nc.scalar.activation(pt[:], xt[:], mybir.ActivationFunctionType.Sigmoid)
                # q = sigmoid(-p)
                nc.scalar.activation(
                    q[:, sl], pt[:], mybir.ActivationFunctionType.Sigmoid, scale=-1.0
                )
            for i in range(n_tiles):
                sl = bass.ts(i, TILE_W)
                nlt = tmp.tile([P, TILE_W], mybir.dt.float32, tag="nl")
                nc.scalar.activation(nlt[:], q[:, sl], mybir.ActivationFunctionType.Ln)
                lt = io.tile([P, TILE_W], mybir.dt.float32, tag="l")
                nc.vector.tensor_scalar_mul(lt[:], nlt[:], -1.0)
                nc.sync.dma_start(l[:, sl], lt[:])
    nc.compile()
    return nc


def _get_nc():
    if "nc" not in _CACHE:
        _CACHE["nc"] = _build_nc()
    return _CACHE["nc"]


def run_device(preds, targs, trace=False):
    """Run the SPMD bass kernel on cores 0..7; returns (nl_full, BassKernelResults)."""
    from concourse.bass_utils import run_bass_kernel_spmd

    nc = _get_nc()
    in_maps = []
    for b in range(B):
        in_maps.append({
            "preds": np.ascontiguousarray(preds[b].reshape(P, FREE), dtype=np.float32),
        })
    res = run_bass_kernel_spmd(nc, in_maps, core_ids=list(range(B)), trace=trace)
    nl = np.stack([res.results[b]["loss"] for b in range(B)])
    return nl.reshape(B, C, D, H, W), res


def _host_finish(preds, targs, nl):
    """Mirror of the reference's host-side get_idxs/pad + dice/mean reductions."""
    x = np.asarray(preds).reshape(-1)
    t = np.asarray(targs).reshape(-1)
    l = np.asarray(nl).reshape(-1).copy()
    numel = t.size
    n_pos = int(t.sum())
    n_neg = numel - n_pos
    if n_pos == 0:
        n_hns = int(DEFAULT_NEG_PERC * n_neg)
    else:
        n_hns = min(n_pos * OHNM_RATIO, n_neg)

    pos_idxs = np.nonzero(t == 1)[0]
    # patch positives: loss|t=1 = softplus(p) - p = nl - p
    p_pos = 1.0 / (1.0 + np.exp(-x[pos_idxs].astype(np.float64)))
    l[pos_idxs] = l[pos_idxs] - p_pos.astype(np.float32)

    neg_losses = l[t == 0]
    if n_hns > 0:
        if n_hns < neg_losses.size:
            part = np.argpartition(-neg_losses, n_hns - 1)[:n_hns]
        else:
            part = np.arange(neg_losses.size)
        hns_idxs = part[np.argsort(-neg_losses[part])]
    else:
        hns_idxs = np.empty(0, dtype=np.int64)
    idxs = np.concatenate([hns_idxs, pos_idxs]).astype(np.int64)
    n_needed = len(idxs) % (B * C)
    if n_needed != 0:
        mask = np.ones(numel, dtype=bool)
        mask[idxs] = False
        remaining = np.nonzero(mask)[0]
        w = remaining.astype(np.float64)
        rng = np.random.default_rng(0)
        extra = rng.choice(remaining, size=n_needed, replace=False, p=w / w.sum())
        idxs = np.concatenate([idxs, extra.astype(np.int64)])

    x_sel = x[idxs].astype(np.float64)
    p_sel = 1.0 / (1.0 + np.exp(-x_sel))          # sigmoid(preds) at selected
    t_sel = t[idxs].astype(np.float64)
    loss_sel = l[idxs].astype(np.float64)

    p2 = (1.0 / (1.0 + np.exp(-p_sel))).reshape(B * C, -1)   # dice re-sigmoids
    ts = t_sel.reshape(B * C, -1)
    inter = (p2 * ts).sum(axis=1)
    denom = p2.sum(axis=1) + ts.sum(axis=1)
    dice = np.mean(1.0 - (2.0 * inter + EPS) / (denom + EPS))
    return np.float32(dice + loss_sel.mean())


def kernel(preds, targs):
    preds = np.asarray(preds, dtype=np.float32)
    targs = np.asarray(targs, dtype=np.int32)
    assert preds.shape == (B, C, D, H, W) and targs.shape == (B, C, D, H, W)
    nl, _ = run_device(preds, targs, trace=False)
    return _host_finish(preds, targs, nl)


# revision 6
# speedup vs baseline: 1.6990x; 1.4882x over previous
"""DiceBCE + OHNM loss for Trainium2 (8 NeuronCores, SPMD data-parallel over batch).

Device side (Bass/Tile, one launch, core b handles batch element b):
  reads preds[b] (f32), computes p = sigmoid(x) — the normalization the
  reference applies before BCE and the quantity whose descending order IS the
  descending order of the negative-class BCE loss (loss|t=0 = softplus(p),
  strictly increasing) — and writes p back to HBM as fp16.

Host side (data-dependent glue, mirrors the reference's host-side numpy):
  top-k hard-negative selection (descending p), positive gather, seeded-RNG
  padding, then the loss values softplus(p) / softplus(-p) and the tiny
  dice + mean reductions over the ~336k selected elements.
"""

import numpy as np

B, C, D, H, W = 8, 1, 128, 128, 128
P = 128
FREE = (C * D * H * W) // P  # 16384 elements per partition per core
N_TILES = 4
TILE_W = FREE // N_TILES  # 4096
EPS = 1e-10
OHNM_RATIO = 3
DEFAULT_NEG_PERC = 0.1

_CACHE = {}


def _build_nc():
    """Raw-Bass (no TileContext — saves the kernel-tail drain/barrier ~7us).

    All 4 input tiles + 4 output tiles stay resident in SBUF (96KB/partition),
    so there is no buffer reuse and the semaphore protocol is trivial:
      sync:   issue the 4 input DMAs back-to-back (inputs get HBM priority),
              then issue each output DMA as its sigmoid completes,
              then wait for all output DMAs to land.
      scalar: per tile, wait for its input DMA, run one fp32->fp16 Sigmoid.
    """
    import contextlib

    from concourse import bacc, mybir

    nc = bacc.Bacc("TRN2", target_bir_lowering=False, debug=False, num_devices=B)
    x = nc.dram_tensor("preds", [P, FREE], mybir.dt.float32, kind="ExternalInput").ap()
    po = nc.dram_tensor("p", [P, FREE], mybir.dt.float16, kind="ExternalOutput").ap()

    with contextlib.ExitStack() as ctx:
        xts = [ctx.enter_context(nc.sbuf_tensor(f"xt{i}", [P, TILE_W], mybir.dt.float32))
               for i in range(N_TILES)]
        pts = [ctx.enter_context(nc.sbuf_tensor(f"pt{i}", [P, TILE_W], mybir.dt.float16))
               for i in range(N_TILES)]
        in_sem = ctx.enter_context(nc.semaphore("in_sem"))
        act_sem = ctx.enter_context(nc.semaphore("act_sem"))
        out_sem = ctx.enter_context(nc.semaphore("out_sem"))
        block = ctx.enter_context(nc.Block())

        @block.sync
        def _(sync):
            for i in range(N_TILES):
                sync.dma_start(
                    xts[i][:, :], x[:, i * TILE_W:(i + 1) * TILE_W]
                ).then_inc(in_sem, 16)
            for i in range(N_TILES):
                sync.wait_ge(act_sem, i + 1)
                sync.dma_start(
                    po[:, i * TILE_W:(i + 1) * TILE_W], pts[i][:, :]
                ).then_inc(out_sem, 16)
            sync.wait_ge(out_sem, N_TILES * 16)

        @block.scalar
        def _(scalar):
            for i in range(N_TILES):
                scalar.wait_ge(in_sem, (i + 1) * 16)
                nc.scalar.activation(
                    pts[i][:, :], xts[i][:, :], mybir.ActivationFunctionType.Sigmoid
                ).then_inc(act_sem, 1)
    nc.compile()
    return nc


def _get_nc():
    if "nc" not in _CACHE:
        _CACHE["nc"] = _build_nc()
    return _CACHE["nc"]


def run_device(preds, targs=None, trace=False, nc=None):
    """Run the SPMD bass kernel on cores 0..7; returns (p_full, BassKernelResults)."""
    from concourse.bass_utils import run_bass_kernel_spmd

    if nc is None:
        nc = _get_nc()
    in_maps = []
    for b in range(B):
        in_maps.append({
            "preds": np.ascontiguousarray(preds[b].reshape(P, FREE), dtype=np.float32),
        })
    res = run_bass_kernel_spmd(nc, in_maps, core_ids=list(range(B)), trace=trace)
    p = np.stack([res.results[b]["p"] for b in range(B)])
    return p.reshape(B, C, D, H, W), res


def _host_finish(preds, targs, pmap):
    """Mirror of the reference's host-side get_idxs/pad + dice/mean reductions."""
    x = np.asarray(preds).reshape(-1)
    t = np.asarray(targs).reshape(-1)
    pf = np.asarray(pmap).reshape(-1)
    numel = t.size
    n_pos = int(t.sum())
    n_neg = numel - n_pos
    if n_pos == 0:
        n_hns = int(DEFAULT_NEG_PERC * n_neg)
    else:
        n_hns = min(n_pos * OHNM_RATIO, n_neg)

    # rank negatives: descending loss == descending p  (loss|t=0 = softplus(p))
    neg_p = pf[t == 0]
    if n_hns > 0:
        if n_hns < neg_p.size:
            part = np.argpartition(-neg_p, n_hns - 1)[:n_hns]
        else:
            part = np.arange(neg_p.size)
        hns_idxs = part[np.argsort(-neg_p[part], kind="stable")]
    else:
        hns_idxs = np.empty(0, dtype=np.int64)
    pos_idxs = np.nonzero(t == 1)[0]
    idxs = np.concatenate([hns_idxs, pos_idxs]).astype(np.int64)
    n_needed = len(idxs) % (B * C)
    if n_needed != 0:
        mask = np.ones(numel, dtype=bool)
        mask[idxs] = False
        remaining = np.nonzero(mask)[0]
        w = remaining.astype(np.float64)
        rng = np.random.default_rng(0)
        extra = rng.choice(remaining, size=n_needed, replace=False, p=w / w.sum())
        idxs = np.concatenate([idxs, extra.astype(np.int64)])

    x_sel = x[idxs].astype(np.float64)
    p_sel = 1.0 / (1.0 + np.exp(-x_sel))          # sigmoid(preds) at selected, exact
    t_sel = t[idxs].astype(np.float64)
    # loss at selected sites: t=0 -> softplus(p) from the device map (the map
    # the ranking ran on); t=1 -> softplus(-p) exact from x
    pq_sel = pf[idxs].astype(np.float64)
    loss_sel = np.where(
        t_sel == 0, np.log1p(np.exp(pq_sel)), np.log1p(np.exp(-p_sel))
    )

    p2 = (1.0 / (1.0 + np.exp(-p_sel))).reshape(B * C, -1)   # dice re-sigmoids
    ts = t_sel.reshape(B * C, -1)
    inter = (p2 * ts).sum(axis=1)
    denom = p2.sum(axis=1) + ts.sum(axis=1)
    dice = np.mean(1.0 - (2.0 * inter + EPS) / (denom + EPS))
    return np.float32(dice + loss_sel.mean())


def kernel(preds, targs):
    preds = np.asarray(preds, dtype=np.float32)
    targs = np.asarray(targs, dtype=np.int32)
    assert preds.shape == (B, C, D, H, W) and targs.shape == (B, C, D, H, W)
    pmap, _ = run_device(preds, trace=False)
    return _host_finish(preds, targs, pmap)


# revision 7
# speedup vs baseline: 2.0543x; 1.2092x over previous
"""DiceBCE + OHNM loss for Trainium2 (8 NeuronCores, SPMD data-parallel over batch).

Device side (Bass/Tile, one launch, core b handles batch element b):
  reads preds[b] (f32), computes p = sigmoid(x) — the normalization the
  reference applies before BCE and the quantity whose descending order IS the
  descending order of the negative-class BCE loss (loss|t=0 = softplus(p),
  strictly increasing) — and writes p back to HBM as fp16.

Host side (data-dependent glue, mirrors the reference's host-side numpy):
  top-k hard-negative selection (descending p), positive gather, seeded-RNG
  padding, then the loss values softplus(p) / softplus(-p) and the tiny
  dice + mean reductions over the ~336k selected elements.
"""

import numpy as np

B, C, D, H, W = 8, 1, 128, 128, 128
P = 128
FREE = (C * D * H * W) // P  # 16384 elements per partition per core
N_TILES = 4
TILE_W = FREE // N_TILES  # 4096
EPS = 1e-10
OHNM_RATIO = 3
DEFAULT_NEG_PERC = 0.1

_CACHE = {}


def _build_nc():
    """Raw-Bass (no TileContext — saves the kernel-tail drain/barrier ~7us).

    All 4 input tiles + 4 output tiles stay resident in SBUF (96KB/partition),
    so there is no buffer reuse and the semaphore protocol is trivial:
      sync:   issue the 4 input DMAs back-to-back (inputs get HBM priority),
              then issue each output DMA as its sigmoid completes,
              then wait for all output DMAs to land.
      scalar: per tile, wait for its input DMA, run one fp32->fp16 Sigmoid.
    """
    import contextlib

    from concourse import bacc, mybir

    nc = bacc.Bacc("TRN2", target_bir_lowering=False, debug=False, num_devices=B)
    x = nc.dram_tensor("preds", [P, FREE], mybir.dt.float32, kind="ExternalInput").ap()
    po = nc.dram_tensor("p", [P, FREE], mybir.dt.float16, kind="ExternalOutput").ap()

    with contextlib.ExitStack() as ctx:
        xts = [ctx.enter_context(nc.sbuf_tensor(f"xt{i}", [P, TILE_W], mybir.dt.float32))
               for i in range(N_TILES)]
        pts = [ctx.enter_context(nc.sbuf_tensor(f"pt{i}", [P, TILE_W], mybir.dt.float16))
               for i in range(N_TILES)]
        in_sem = ctx.enter_context(nc.semaphore("in_sem"))
        act_sem = ctx.enter_context(nc.semaphore("act_sem"))
        out_sem = ctx.enter_context(nc.semaphore("out_sem"))
        block = ctx.enter_context(nc.Block())

        @block.sync
        def _(sync):
            for i in range(N_TILES):
                sync.dma_start(
                    xts[i][:, :], x[:, i * TILE_W:(i + 1) * TILE_W]
                ).then_inc(in_sem, 16)
            for i in range(N_TILES):
                sync.wait_ge(act_sem, i + 1)
                sync.dma_start(
                    po[:, i * TILE_W:(i + 1) * TILE_W], pts[i][:, :]
                ).then_inc(out_sem, 16)
            sync.wait_ge(out_sem, N_TILES * 16)

        @block.scalar
        def _(scalar):
            for i in range(N_TILES):
                scalar.wait_ge(in_sem, (i + 1) * 16)
                nc.scalar.activation(
                    pts[i][:, :], xts[i][:, :], mybir.ActivationFunctionType.Sigmoid
                ).then_inc(act_sem, 1)
    nc.compile()
    return nc


def _get_nc():
    if "nc" not in _CACHE:
        _CACHE["nc"] = _build_nc()
    return _CACHE["nc"]


def run_device(preds, targs=None, trace=False, nc=None):
    """Run the SPMD bass kernel on cores 0..7; returns (p_full, BassKernelResults)."""
    from concourse.bass_utils import run_bass_kernel_spmd

    if nc is None:
        nc = _get_nc()
    in_maps = []
    for b in range(B):
        in_maps.append({
            "preds": np.ascontiguousarray(preds[b].reshape(P, FREE), dtype=np.float32),
        })
    try:
        res = run_bass_kernel_spmd(nc, in_maps, core_ids=list(range(B)), trace=trace)
    except Exception:
        # transient device faults (e.g. NRT_EXEC_UNIT_UNRECOVERABLE) usually
        # clear after the runtime resets the cores; one retry is cheap
        import time
        time.sleep(30)
        res = run_bass_kernel_spmd(nc, in_maps, core_ids=list(range(B)), trace=trace)
    p = np.stack([res.results[b]["p"] for b in range(B)])
    return p.reshape(B, C, D, H, W), res


def _host_finish(preds, targs, pmap):
    """Mirror of the reference's host-side get_idxs/pad + dice/mean reductions."""
    x = np.asarray(preds).reshape(-1)
    t = np.asarray(targs).reshape(-1)
    pf = np.asarray(pmap).reshape(-1)
    numel = t.size
    n_pos = int(t.sum())
    n_neg = numel - n_pos
    if n_pos == 0:
        n_hns = int(DEFAULT_NEG_PERC * n_neg)
    else:
        n_hns = min(n_pos * OHNM_RATIO, n_neg)

    # rank negatives: descending loss == descending p  (loss|t=0 = softplus(p))
    neg_p = pf[t == 0]
    if n_hns > 0:
        if n_hns < neg_p.size:
            part = np.argpartition(-neg_p, n_hns - 1)[:n_hns]
        else:
            part = np.arange(neg_p.size)
        hns_idxs = part[np.argsort(-neg_p[part], kind="stable")]
    else:
        hns_idxs = np.empty(0, dtype=np.int64)
    pos_idxs = np.nonzero(t == 1)[0]
    idxs = np.concatenate([hns_idxs, pos_idxs]).astype(np.int64)
    n_needed = len(idxs) % (B * C)
    if n_needed != 0:
        mask = np.ones(numel, dtype=bool)
        mask[idxs] = False
        remaining = np.nonzero(mask)[0]
        w = remaining.astype(np.float64)
        rng = np.random.default_rng(0)
        extra = rng.choice(remaining, size=n_needed, replace=False, p=w / w.sum())
        idxs = np.concatenate([idxs, extra.astype(np.int64)])

    x_sel = x[idxs].astype(np.float64)
    p_sel = 1.0 / (1.0 + np.exp(-x_sel))          # sigmoid(preds) at selected, exact
    t_sel = t[idxs].astype(np.float64)
    # loss at selected sites: t=0 -> softplus(p) from the device map (the map
    # the ranking ran on); t=1 -> softplus(-p) exact from x
    pq_sel = pf[idxs].astype(np.float64)
    loss_sel = np.where(
        t_sel == 0, np.log1p(np.exp(pq_sel)), np.log1p(np.exp(-p_sel))
    )

    p2 = (1.0 / (1.0 + np.exp(-p_sel))).reshape(B * C, -1)   # dice re-sigmoids
    ts = t_sel.reshape(B * C, -1)
    inter = (p2 * ts).sum(axis=1)
    denom = p2.sum(axis=1) + ts.sum(axis=1)
    dice = np.mean(1.0 - (2.0 * inter + EPS) / (denom + EPS))
    return np.float32(dice + loss_sel.mean())


def kernel(preds, targs):
    preds = np.asarray(preds, dtype=np.float32)
    targs = np.asarray(targs, dtype=np.int32)
    assert preds.shape == (B, C, D, H, W) and targs.shape == (B, C, D, H, W)
    pmap, _ = run_device(preds, trace=False)
    return _host_finish(preds, targs, pmap)


# revision 8
# speedup vs baseline: 2.0860x; 1.0154x over previous
"""DiceBCE + OHNM loss for Trainium2 (8 NeuronCores, SPMD data-parallel over batch).

Device side (Bass/Tile, one launch, core b handles batch element b):
  reads preds[b] (f32), computes p = sigmoid(x) — the normalization the
  reference applies before BCE and the quantity whose descending order IS the
  descending order of the negative-class BCE loss (loss|t=0 = softplus(p),
  strictly increasing) — and writes p back to HBM as fp16.

Host side (data-dependent glue, mirrors the reference's host-side numpy):
  top-k hard-negative selection (descending p), positive gather, seeded-RNG
  padding, then the loss values softplus(p) / softplus(-p) and the tiny
  dice + mean reductions over the ~336k selected elements.
"""

import numpy as np

B, C, D, H, W = 8, 1, 128, 128, 128
P = 128
FREE = (C * D * H * W) // P  # 16384 elements per partition per core
N_TILES = 4
TILE_W = FREE // N_TILES  # 4096
EPS = 1e-10
OHNM_RATIO = 3
DEFAULT_NEG_PERC = 0.1

_CACHE = {}


def _build_nc():
    """Raw-Bass (no TileContext — saves the kernel-tail drain/barrier ~7us).

    All 4 input tiles + 4 output tiles stay resident in SBUF (96KB/partition),
    so there is no buffer reuse and the semaphore protocol is trivial:
      sync:   issue the 4 input DMAs back-to-back (inputs get HBM priority),
              then issue each output DMA as its sigmoid completes,
              then wait for all output DMAs to land.
      scalar: per tile, wait for its input DMA, run one fp32->fp16 Sigmoid.
    """
    import contextlib

    from concourse import bacc, mybir

    nc = bacc.Bacc("TRN2", target_bir_lowering=False, debug=False, num_devices=B)
    x = nc.dram_tensor("preds", [P, FREE], mybir.dt.float32, kind="ExternalInput").ap()
    po = nc.dram_tensor("p", [P, FREE], mybir.dt.float16, kind="ExternalOutput").ap()

    with contextlib.ExitStack() as ctx:
        xts = [ctx.enter_context(nc.sbuf_tensor(f"xt{i}", [P, TILE_W], mybir.dt.float32))
               for i in range(N_TILES)]
        pts = [ctx.enter_context(nc.sbuf_tensor(f"pt{i}", [P, TILE_W], mybir.dt.float16))
               for i in range(N_TILES)]
        in_sem = ctx.enter_context(nc.semaphore("in_sem"))
        act_sem = ctx.enter_context(nc.semaphore("act_sem"))
        out_sem = ctx.enter_context(nc.semaphore("out_sem"))
        block = ctx.enter_context(nc.Block())

        @block.sync
        def _(sync):
            for i in range(N_TILES):
                sync.dma_start(
                    xts[i][:, :], x[:, i * TILE_W:(i + 1) * TILE_W]
                ).then_inc(in_sem, 16)
            for i in range(N_TILES):
                sync.wait_ge(act_sem, i + 1)
                sync.dma_start(
                    po[:, i * TILE_W:(i + 1) * TILE_W], pts[i][:, :]
                ).then_inc(out_sem, 16)
            sync.wait_ge(out_sem, N_TILES * 16)

        @block.scalar
        def _(scalar):
            for i in range(N_TILES):
                scalar.wait_ge(in_sem, (i + 1) * 16)
                nc.scalar.activation(
                    pts[i][:, :], xts[i][:, :], mybir.ActivationFunctionType.Sigmoid
                ).then_inc(act_sem, 1)
    nc.compile()
    return nc


def _get_nc():
    if "nc" not in _CACHE:
        _CACHE["nc"] = _build_nc()
    return _CACHE["nc"]


def run_device(preds, targs=None, trace=False, nc=None):
    """Run the SPMD bass kernel on cores 0..7; returns (p_full, BassKernelResults)."""
    from concourse.bass_utils import run_bass_kernel_spmd

    if nc is None:
        nc = _get_nc()
    in_maps = []
    for b in range(B):
        in_maps.append({
            "preds": np.ascontiguousarray(preds[b].reshape(P, FREE), dtype=np.float32),
        })
    try:
        res = run_bass_kernel_spmd(nc, in_maps, core_ids=list(range(B)), trace=trace)
    except Exception:
        # transient device faults (e.g. NRT_EXEC_UNIT_UNRECOVERABLE) usually
        # clear after the runtime resets the cores; one retry is cheap
        import time
        time.sleep(30)
        res = run_bass_kernel_spmd(nc, in_maps, core_ids=list(range(B)), trace=trace)
    p = np.stack([res.results[b]["p"] for b in range(B)])
    return p.reshape(B, C, D, H, W), res


def _host_finish(preds, targs, pmap):
    """Mirror of the reference's host-side get_idxs/pad + dice/mean reductions."""
    x = np.asarray(preds).reshape(-1)
    t = np.asarray(targs).reshape(-1)
    pf = np.asarray(pmap).reshape(-1)
    numel = t.size
    n_pos = int(t.sum())
    n_neg = numel - n_pos
    if n_pos == 0:
        n_hns = int(DEFAULT_NEG_PERC * n_neg)
    else:
        n_hns = min(n_pos * OHNM_RATIO, n_neg)

    # rank negatives: descending loss == descending p == descending x
    # (loss|t=0 = softplus(p), p = sigmoid(x), both strictly increasing).
    # Sorting by x equals sorting by the device fp16 p-map with x breaking the
    # quantization ties, and reproduces the reference's f32-loss order exactly
    # up to f32 rounding ties.
    neg_x = x[t == 0]
    if n_hns > 0:
        if n_hns < neg_x.size:
            part = np.argpartition(-neg_x, n_hns - 1)[:n_hns]
        else:
            part = np.arange(neg_x.size)
        hns_idxs = part[np.argsort(-neg_x[part], kind="stable")]
    else:
        hns_idxs = np.empty(0, dtype=np.int64)
    pos_idxs = np.nonzero(t == 1)[0]
    idxs = np.concatenate([hns_idxs, pos_idxs]).astype(np.int64)
    n_needed = len(idxs) % (B * C)
    if n_needed != 0:
        mask = np.ones(numel, dtype=bool)
        mask[idxs] = False
        remaining = np.nonzero(mask)[0]
        w = remaining.astype(np.float64)
        rng = np.random.default_rng(0)
        extra = rng.choice(remaining, size=n_needed, replace=False, p=w / w.sum())
        idxs = np.concatenate([idxs, extra.astype(np.int64)])

    x_sel = x[idxs].astype(np.float64)
    p_sel = 1.0 / (1.0 + np.exp(-x_sel))          # sigmoid(preds) at selected, exact
    t_sel = t[idxs].astype(np.float64)
    # loss at selected sites: t=0 -> softplus(p) from the device map (the map
    # the ranking ran on); t=1 -> softplus(-p) exact from x
    pq_sel = pf[idxs].astype(np.float64)
    loss_sel = np.where(
        t_sel == 0, np.log1p(np.exp(pq_sel)), np.log1p(np.exp(-p_sel))
    )

    p2 = (1.0 / (1.0 + np.exp(-p_sel))).reshape(B * C, -1)   # dice re-sigmoids
    ts = t_sel.reshape(B * C, -1)
    inter = (p2 * ts).sum(axis=1)
    denom = p2.sum(axis=1) + ts.sum(axis=1)
    dice = np.mean(1.0 - (2.0 * inter + EPS) / (denom + EPS))
    return np.float32(dice + loss_sel.mean())


def kernel(preds, targs):
    preds = np.asarray(preds, dtype=np.float32)
    targs = np.asarray(targs, dtype=np.int32)
    assert preds.shape == (B, C, D, H, W) and targs.shape == (B, C, D, H, W)
    pmap, _ = run_device(preds, trace=False)
    return _host_finish(preds, targs, pmap)
